# revision 16
# baseline (speedup 1.0000x reference)
"""Trainium2 Bass kernel for additive (Bahdanau) attention.

reference:
    proj_f = features @ W1_w + W1_b          # [B, L, ATT]
    proj_h = (hidden @ W2_w + W2_b)[:, None] # [B, 1, ATT]
    scores = tanh(proj_f + proj_h) @ V_w + V_b   # [B, L]
    alpha  = softmax(scores, axis=1)
    context = einsum('bl,ble->be', alpha, features)
    returns (alpha, context)

Sharding: data-parallel over batch B=64 across 8 cores (8 examples/core).
Weights replicated. No collectives.

Per-core algorithm (X = 8 examples):
  - cast-DMA features f32 -> bf16 DRAM scratch (gpsimd SWDGE).
  - HW transpose-DMA bf16 DRAM -> SBUF: fT [ENC_chunk=128, L=1024] x8.
  - main matmul in [ATT_part, L_free] orientation: lhsT = W1 chunk
    (natural layout), rhs = fT. PSUM [128, 512] f32.
  - ACT applies tanh fused with per-partition bias = (W1_b + W2_b +
    hidden @ W2_w) transposed - computed in a small f32 prepass.
  - V-dot on PE: scores[1, L] += V_chunk[128,1].T @ tanh_tile, accumulated
    over ATT chunks in PSUM.  (V_b dropped: softmax is shift-invariant.)
  - softmax per example on DVE/ACT (free-dim reduces on [1, 1024]).
  - context on DVE: tensor_tensor_reduce over fT tiles with alpha
    replicated across partitions (gpsimd partition_broadcast).
"""

import numpy as np

B, L, ENC, DEC, ATT = 64, 1024, 1024, 1024, 1024
N_CORES = 8
X = B // N_CORES  # examples per core
P = 128
NE = ENC // P  # 8
NA = ATT // P  # 8
ND = DEC // P  # 8
LH = 512       # free-dim half for fp32 PSUM bank
NL = L // LH   # 2

_CACHE = {}


def _build():
    import concourse.bacc as bacc
    import concourse.mybir as mybir
    import concourse.tile as tile

    f32, bf16 = mybir.dt.float32, mybir.dt.bfloat16
    Tanh = mybir.ActivationFunctionType.Tanh
    Exp = mybir.ActivationFunctionType.Exp
    add = mybir.AluOpType.add
    mult = mybir.AluOpType.mult
    AX = mybir.AxisListType.X

    nc = bacc.Bacc("TRN2", target_bir_lowering=False, debug=False, num_devices=N_CORES)

    feats = nc.declare_dram_parameter("features", [X, L, ENC], f32, isOutput=False)
    hid = nc.declare_dram_parameter("hidden_state", [X, DEC], f32, isOutput=False)
    w1 = nc.declare_dram_parameter("W1_w", [ENC, ATT], f32, isOutput=False)
    w1b = nc.declare_dram_parameter("W1_b", [ATT], f32, isOutput=False)
    w2 = nc.declare_dram_parameter("W2_w", [DEC, ATT], f32, isOutput=False)
    w2b = nc.declare_dram_parameter("W2_b", [ATT], f32, isOutput=False)
    vw = nc.declare_dram_parameter("V_w", [ATT], f32, isOutput=False)
    alpha_o = nc.declare_dram_parameter("alpha", [X, L], f32, isOutput=True)
    ctx_o = nc.declare_dram_parameter("context", [X, ENC], f32, isOutput=True)

    eye_dram = nc.inline_tensor(np.eye(P, dtype=np.float32), "eye128")

    with tile.TileContext(nc) as tc:
        with (
            tc.tile_pool(name="const", bufs=1) as const,
            tc.tile_pool(name="dram", bufs=4, space="DRAM") as dram,
            tc.tile_pool(name="ft", bufs=3 * NE + 4) as ftp,
            tc.tile_pool(name="mm", bufs=3, space="PSUM") as psum,
            tc.tile_pool(name="sc", bufs=2, space="PSUM") as spsum,
            tc.tile_pool(name="ct", bufs=1, space="PSUM") as ctpsum,
            tc.tile_pool(name="tb", bufs=6) as tp,
            tc.tile_pool(name="jk", bufs=2) as jp,
            tc.tile_pool(name="al", bufs=2) as alp,
            tc.tile_pool(name="ms", bufs=1) as ms,
        ):
            # ---------------- prep: constants & weights ----------------
            eye = const.tile([P, P], f32, tag="eye")
            nc.sync.dma_start(eye[:], eye_dram[:, :])

            w1bf = []
            for e in range(NE):
                t = const.tile([P, ATT], bf16, tag=f"w1_{e}")
                nc.gpsimd.dma_start(t[:], w1[P * e : P * (e + 1), :])
                w1bf.append(t)

            w2t = []
            for e in range(ND):
                t = const.tile([P, ATT], bf16, tag=f"w2_{e}")
                nc.gpsimd.dma_start(t[:], w2[P * e : P * (e + 1), :])
                w2t.append(t)

            # hT_all[p, c, x] = hid[x, 128c + p]  (fine-grained strided DMA)
            hT = ms.tile([P, ND, X], f32, tag="hT")
            hid_t = hid.rearrange("x (c p) -> p c x", p=P)
            for c in range(ND):
                nc.gpsimd.dma_start(hT[:, c, :], hid_t[:, c, :])
            hTb = ms.tile([P, ND, X], bf16, tag="hTb")
            nc.vector.tensor_copy(hTb[:], hT[:])

            # bias vectors transposed: bT[p, c] = v[128c + p]
            w1bT = ms.tile([P, NA], f32, tag="w1bT")
            nc.gpsimd.dma_start(w1bT[:], w1b.rearrange("(c p) -> p c", p=P))
            w2bT = ms.tile([P, NA], f32, tag="w2bT")
            nc.gpsimd.dma_start(w2bT[:], w2b.rearrange("(c p) -> p c", p=P))
            vwT = ms.tile([P, NA], f32, tag="vwT")
            nc.gpsimd.dma_start(vwT[:], vw.rearrange("(c p) -> p c", p=P))
            vwbf = ms.tile([P, NA], bf16, tag="vwbf")
            nc.vector.tensor_copy(vwbf[:], vwT[:])

            bT = ms.tile([P, NA], f32, tag="bT")
            nc.vector.tensor_add(bT[:], w1bT[:], w2bT[:])

            # proj_h transposed, plus bias: phb[p, a, x]
            phb = ms.tile([P, NA, X], f32, tag="phb")
            for a in range(NA):
                ph_ps = psum.tile([P, X], f32, tag="mm")
                for e in range(ND):
                    nc.tensor.matmul(
                        ph_ps[:],
                        w2t[e][:, P * a : P * (a + 1)],
                        hTb[:, e, :],
                        start=(e == 0),
                        stop=(e == ND - 1),
                    )
                nc.vector.tensor_scalar_add(phb[:, a, :], ph_ps[:], bT[:, a : a + 1])

            # ---------------- outputs accumulated in SBUF ----------------
            ctx_sb = ms.tile([P, NE * X], f32, tag="ctx_sb")

            # ---------------- main per-example pipeline ----------------
            # V-dot matmuls are delayed by one (a, lh) block so the PE never
            # waits on the ACT tanh of the block it just produced.
            pending = []

            def flush_pending():
                for sc_ap, vw_ap, tb_ap, st, sp in pending:
                    nc.tensor.matmul(sc_ap, vw_ap, tb_ap, start=st, stop=sp)
                pending.clear()

            def emit_cast(x):
                fbf = dram.tile([L, ENC], bf16, tag="fbf")
                for c in range(8):
                    nc.gpsimd.dma_start(
                        fbf[P * c : P * (c + 1), :], feats[x, P * c : P * (c + 1), :]
                    )
                return fbf

            def emit_transpose(fbf):
                # HW transpose-DMA: fT[e] = fbf[:, 128e:128(e+1)].T
                fts = []
                for e in range(NE):
                    ft = ftp.tile([P, L], bf16, tag="ft")
                    nc.sync.dma_start(ft[:], fbf[:, P * e : P * (e + 1)], transpose=True)
                    fts.append(ft)
                return fts

            # prologue: stage examples 0-2
            fbfs = {0: emit_cast(0), 1: emit_cast(1), 2: emit_cast(2)}
            ft_map = {0: emit_transpose(fbfs[0]), 1: emit_transpose(fbfs[1])}

            for x in range(X):
                if x + 3 < X:
                    fbfs[x + 3] = emit_cast(x + 3)
                if x + 2 < X:
                    ft_map[x + 2] = emit_transpose(fbfs[x + 2])
                fts = ft_map.pop(x)

                sc_ps = spsum.tile([1, L], f32, tag="sc")
                for a in range(NA):
                    for lh in range(NL):
                        pp = psum.tile([P, LH], f32, tag="mm")
                        for e in range(NE):
                            nc.tensor.matmul(
                                pp[:],
                                w1bf[e][:, P * a : P * (a + 1)],
                                fts[e][:, LH * lh : LH * (lh + 1)],
                                start=(e == 0),
                                stop=(e == NE - 1),
                            )
                        flush_pending()
                        tb = tp.tile([P, LH], bf16, tag="tb")
                        nc.scalar.activation(tb[:], pp[:], Tanh, bias=phb[:, a, x : x + 1])
                        pending.append(
                            (
                                sc_ps[:, LH * lh : LH * (lh + 1)],
                                vwbf[:, a : a + 1],
                                tb[:],
                                a == 0,
                                a == NA - 1,
                            )
                        )

                flush_pending()
                # softmax over L on partition 0
                negm = alp.tile([1, 1], f32, tag="negm")
                nc.vector.tensor_reduce(
                    negm[:], sc_ps[:], axis=AX, op=mybir.AluOpType.max, negate=True
                )
                esb = alp.tile([1, L], f32, tag="esb")
                ssum = alp.tile([1, 1], f32, tag="ssum")
                nc.scalar.activation(
                    esb[:], sc_ps[:], Exp, bias=negm[:], accum_out=ssum[:]
                )
                rinv = alp.tile([1, 1], f32, tag="rinv")
                nc.vector.reciprocal(rinv[:], ssum[:])
                a32 = alp.tile([1, L], f32, tag="a32")
                nc.vector.tensor_scalar_mul(a32[:], esb[:], rinv[:])
                nc.sync.dma_start(alpha_o[x, :], a32[:])
                abf = alp.tile([1, L], bf16, tag="abf")
                nc.vector.tensor_scalar_mul(abf[:], esb[:], rinv[:])
                arep = alp.tile([P, L], bf16, tag="arep")
                nc.gpsimd.partition_broadcast(arep[:], abf[:])

                # context: ctx[e-chunk] = sum_l fT[e][:, l] * alpha[l]
                for e in range(NE):
                    jk = jp.tile([P, L], f32, tag="jk")
                    nc.vector.scalar_tensor_tensor(
                        out=jk[:],
                        in0=fts[e][:],
                        scalar=1.0,
                        in1=arep[:],
                        op0=mult,
                        op1=mult,
                        accum_out=ctx_sb[:, X * e + x : X * e + x + 1],
                    )

            # ---------------- epilogue: outputs ----------------
            out_sb = ms.tile([X, ENC], f32, tag="out_sb")
            for e in range(NE):
                ct_ps = ctpsum.tile([X, P], f32, tag="ctps")
                nc.tensor.transpose(ct_ps[:], ctx_sb[:, X * e : X * (e + 1)], eye[:])
                nc.vector.tensor_copy(out_sb[:, P * e : P * (e + 1)], ct_ps[:])
            nc.sync.dma_start(ctx_o[:, :], out_sb[:])

    nc.compile()
    return nc


def kernel(features, hidden_state, W1_w, W1_b, W2_w, W2_b, V_w, V_b):
    from concourse.bass_utils import run_bass_kernel_spmd

    if "nc" not in _CACHE:
        _CACHE["nc"] = _build()
    nc = _CACHE["nc"]

    features = np.ascontiguousarray(np.asarray(features, dtype=np.float32))
    hidden_state = np.ascontiguousarray(np.asarray(hidden_state, dtype=np.float32))
    W1_w = np.ascontiguousarray(np.asarray(W1_w, dtype=np.float32))
    W1_b = np.ascontiguousarray(np.asarray(W1_b, dtype=np.float32))
    W2_w = np.ascontiguousarray(np.asarray(W2_w, dtype=np.float32))
    W2_b = np.ascontiguousarray(np.asarray(W2_b, dtype=np.float32))
    V_w = np.ascontiguousarray(np.asarray(V_w, dtype=np.float32))

    in_maps = []
    for c in range(N_CORES):
        in_maps.append(
            {
                "features": np.ascontiguousarray(features[c * X : (c + 1) * X]),
                "hidden_state": np.ascontiguousarray(hidden_state[c * X : (c + 1) * X]),
                "W1_w": W1_w,
                "W1_b": W1_b,
                "W2_w": W2_w,
                "W2_b": W2_b,
                "V_w": V_w,
            }
        )

    res = run_bass_kernel_spmd(nc, in_maps, list(range(N_CORES)), **_CACHE.get("run_kwargs", {}))
    _CACHE["last_result"] = res
    alpha = np.concatenate([res.results[c]["alpha"] for c in range(N_CORES)], axis=0)
    context = np.concatenate([res.results[c]["context"] for c in range(N_CORES)], axis=0)
    return alpha, context


# revision 18
# speedup vs baseline: 1.0074x; 1.0074x over previous
"""Trainium2 Bass kernel for additive (Bahdanau) attention.

reference:
    proj_f = features @ W1_w + W1_b          # [B, L, ATT]
    proj_h = (hidden @ W2_w + W2_b)[:, None] # [B, 1, ATT]
    scores = tanh(proj_f + proj_h) @ V_w + V_b   # [B, L]
    alpha  = softmax(scores, axis=1)
    context = einsum('bl,ble->be', alpha, features)
    returns (alpha, context)

Sharding: data-parallel over batch B=64 across 8 cores (8 examples/core).
Weights replicated. No collectives.

Per-core algorithm (X = 8 examples):
  - cast-DMA features f32 -> bf16 DRAM scratch (gpsimd SWDGE).
  - HW transpose-DMA bf16 DRAM -> SBUF: fT [ENC_chunk=128, L=1024] x8.
  - main matmul in [ATT_part, L_free] orientation: lhsT = W1 chunk
    (natural layout), rhs = fT. PSUM [128, 512] f32.
  - ACT applies tanh fused with per-partition bias = (W1_b + W2_b +
    hidden @ W2_w) transposed - computed in a small f32 prepass.
  - V-dot on PE: scores[1, L] += V_chunk[128,1].T @ tanh_tile, accumulated
    over ATT chunks in PSUM.  (V_b dropped: softmax is shift-invariant.)
  - softmax per example on DVE/ACT (free-dim reduces on [1, 1024]).
  - context on DVE: tensor_tensor_reduce over fT tiles with alpha
    replicated across partitions (gpsimd partition_broadcast).
"""

import numpy as np

B, L, ENC, DEC, ATT = 64, 1024, 1024, 1024, 1024
N_CORES = 8
X = B // N_CORES  # examples per core
P = 128
NE = ENC // P  # 8
NA = ATT // P  # 8
ND = DEC // P  # 8
LH = 512       # free-dim half for fp32 PSUM bank
NL = L // LH   # 2

_CACHE = {}


def _build():
    import concourse.bacc as bacc
    import concourse.mybir as mybir
    import concourse.tile as tile

    f32, bf16 = mybir.dt.float32, mybir.dt.bfloat16
    Tanh = mybir.ActivationFunctionType.Tanh
    Exp = mybir.ActivationFunctionType.Exp
    add = mybir.AluOpType.add
    mult = mybir.AluOpType.mult
    AX = mybir.AxisListType.X

    nc = bacc.Bacc("TRN2", target_bir_lowering=False, debug=False, num_devices=N_CORES)

    feats = nc.declare_dram_parameter("features", [X, L, ENC], f32, isOutput=False)
    hid = nc.declare_dram_parameter("hidden_state", [X, DEC], f32, isOutput=False)
    w1 = nc.declare_dram_parameter("W1_w", [ENC, ATT], f32, isOutput=False)
    w1b = nc.declare_dram_parameter("W1_b", [ATT], f32, isOutput=False)
    w2 = nc.declare_dram_parameter("W2_w", [DEC, ATT], f32, isOutput=False)
    w2b = nc.declare_dram_parameter("W2_b", [ATT], f32, isOutput=False)
    vw = nc.declare_dram_parameter("V_w", [ATT], f32, isOutput=False)
    alpha_o = nc.declare_dram_parameter("alpha", [X, L], f32, isOutput=True)
    ctx_o = nc.declare_dram_parameter("context", [X, ENC], f32, isOutput=True)

    eye_dram = nc.inline_tensor(np.eye(P, dtype=np.float32), "eye128")

    with tile.TileContext(nc) as tc:
        with (
            tc.tile_pool(name="const", bufs=1) as const,
            tc.tile_pool(name="dram", bufs=8, space="DRAM") as dram,
            tc.tile_pool(name="ft", bufs=3 * NE + 4) as ftp,
            tc.tile_pool(name="mm", bufs=3, space="PSUM") as psum,
            tc.tile_pool(name="sc", bufs=2, space="PSUM") as spsum,
            tc.tile_pool(name="ct", bufs=1, space="PSUM") as ctpsum,
            tc.tile_pool(name="tb", bufs=6) as tp,
            tc.tile_pool(name="jk", bufs=2) as jp,
            tc.tile_pool(name="al", bufs=2) as alp,
            tc.tile_pool(name="ms", bufs=1) as ms,
        ):
            # ---------------- prep: constants & weights ----------------
            eye = const.tile([P, P], f32, tag="eye")
            nc.sync.dma_start(eye[:], eye_dram[:, :])

            w1bf = []
            for e in range(NE):
                t = const.tile([P, ATT], bf16, tag=f"w1_{e}")
                nc.gpsimd.dma_start(t[:], w1[P * e : P * (e + 1), :])
                w1bf.append(t)

            w2t = []
            for e in range(ND):
                t = const.tile([P, ATT], bf16, tag=f"w2_{e}")
                nc.gpsimd.dma_start(t[:], w2[P * e : P * (e + 1), :])
                w2t.append(t)

            # hT_all[p, c, x] = hid[x, 128c + p]  (fine-grained strided DMA)
            hT = ms.tile([P, ND, X], f32, tag="hT")
            hid_t = hid.rearrange("x (c p) -> p c x", p=P)
            for c in range(ND):
                nc.gpsimd.dma_start(hT[:, c, :], hid_t[:, c, :])
            hTb = ms.tile([P, ND, X], bf16, tag="hTb")
            nc.vector.tensor_copy(hTb[:], hT[:])

            # bias vectors transposed: bT[p, c] = v[128c + p]
            w1bT = ms.tile([P, NA], f32, tag="w1bT")
            nc.gpsimd.dma_start(w1bT[:], w1b.rearrange("(c p) -> p c", p=P))
            w2bT = ms.tile([P, NA], f32, tag="w2bT")
            nc.gpsimd.dma_start(w2bT[:], w2b.rearrange("(c p) -> p c", p=P))
            vwT = ms.tile([P, NA], f32, tag="vwT")
            nc.gpsimd.dma_start(vwT[:], vw.rearrange("(c p) -> p c", p=P))
            vwbf = ms.tile([P, NA], bf16, tag="vwbf")
            nc.vector.tensor_copy(vwbf[:], vwT[:])

            bT = ms.tile([P, NA], f32, tag="bT")
            nc.vector.tensor_add(bT[:], w1bT[:], w2bT[:])

            # proj_h transposed, plus bias: phb[p, a, x]
            phb = ms.tile([P, NA, X], f32, tag="phb")
            for a in range(NA):
                ph_ps = psum.tile([P, X], f32, tag="mm")
                for e in range(ND):
                    nc.tensor.matmul(
                        ph_ps[:],
                        w2t[e][:, P * a : P * (a + 1)],
                        hTb[:, e, :],
                        start=(e == 0),
                        stop=(e == ND - 1),
                    )
                nc.vector.tensor_scalar_add(phb[:, a, :], ph_ps[:], bT[:, a : a + 1])

            # ---------------- outputs accumulated in SBUF ----------------
            ctx_sb = ms.tile([P, NE * X], f32, tag="ctx_sb")

            # ---------------- main per-example pipeline ----------------
            # V-dot matmuls are delayed by one (a, lh) block so the PE never
            # waits on the ACT tanh of the block it just produced.
            pending = []

            def flush_pending():
                for sc_ap, vw_ap, tb_ap, st, sp in pending:
                    nc.tensor.matmul(sc_ap, vw_ap, tb_ap, start=st, stop=sp)
                pending.clear()

            def emit_cast(x):
                fbf = dram.tile([L, ENC], bf16, tag="fbf")
                for c in range(8):
                    nc.gpsimd.dma_start(
                        fbf[P * c : P * (c + 1), :], feats[x, P * c : P * (c + 1), :]
                    )
                return fbf

            def emit_transpose(fbf):
                # HW transpose-DMA: fT[e] = fbf[:, 128e:128(e+1)].T
                fts = []
                for e in range(NE):
                    ft = ftp.tile([P, L], bf16, tag="ft")
                    nc.sync.dma_start(ft[:], fbf[:, P * e : P * (e + 1)], transpose=True)
                    fts.append(ft)
                return fts

            # prologue: cast everything up-front (casts own the SWDGE queues,
            # transposes the HWDGE queues); transpose 2 examples ahead.
            fbfs = {x: emit_cast(x) for x in range(X)}
            ft_map = {0: emit_transpose(fbfs[0]), 1: emit_transpose(fbfs[1])}

            for x in range(X):
                if x + 2 < X:
                    ft_map[x + 2] = emit_transpose(fbfs[x + 2])
                fts = ft_map.pop(x)

                sc_ps = spsum.tile([1, L], f32, tag="sc")
                for a in range(NA):
                    for lh in range(NL):
                        pp = psum.tile([P, LH], f32, tag="mm")
                        for e in range(NE):
                            nc.tensor.matmul(
                                pp[:],
                                w1bf[e][:, P * a : P * (a + 1)],
                                fts[e][:, LH * lh : LH * (lh + 1)],
                                start=(e == 0),
                                stop=(e == NE - 1),
                            )
                        flush_pending()
                        tb = tp.tile([P, LH], bf16, tag="tb")
                        nc.scalar.activation(tb[:], pp[:], Tanh, bias=phb[:, a, x : x + 1])
                        pending.append(
                            (
                                sc_ps[:, LH * lh : LH * (lh + 1)],
                                vwbf[:, a : a + 1],
                                tb[:],
                                a == 0,
                                a == NA - 1,
                            )
                        )

                flush_pending()
                # softmax over L on partition 0
                negm = alp.tile([1, 1], f32, tag="negm")
                nc.vector.tensor_reduce(
                    negm[:], sc_ps[:], axis=AX, op=mybir.AluOpType.max, negate=True
                )
                esb = alp.tile([1, L], f32, tag="esb")
                ssum = alp.tile([1, 1], f32, tag="ssum")
                nc.scalar.activation(
                    esb[:], sc_ps[:], Exp, bias=negm[:], accum_out=ssum[:]
                )
                rinv = alp.tile([1, 1], f32, tag="rinv")
                nc.vector.reciprocal(rinv[:], ssum[:])
                a32 = alp.tile([1, L], f32, tag="a32")
                nc.vector.tensor_scalar_mul(a32[:], esb[:], rinv[:])
                nc.sync.dma_start(alpha_o[x, :], a32[:])
                abf = alp.tile([1, L], bf16, tag="abf")
                nc.vector.tensor_scalar_mul(abf[:], esb[:], rinv[:])
                arep = alp.tile([P, L], bf16, tag="arep")
                nc.gpsimd.partition_broadcast(arep[:], abf[:])

                # context: ctx[e-chunk] = sum_l fT[e][:, l] * alpha[l]
                for e in range(NE):
                    jk = jp.tile([P, L], f32, tag="jk")
                    nc.vector.scalar_tensor_tensor(
                        out=jk[:],
                        in0=fts[e][:],
                        scalar=1.0,
                        in1=arep[:],
                        op0=mult,
                        op1=mult,
                        accum_out=ctx_sb[:, X * e + x : X * e + x + 1],
                    )

            # ---------------- epilogue: outputs ----------------
            out_sb = ms.tile([X, ENC], f32, tag="out_sb")
            for e in range(NE):
                ct_ps = ctpsum.tile([X, P], f32, tag="ctps")
                nc.tensor.transpose(ct_ps[:], ctx_sb[:, X * e : X * (e + 1)], eye[:])
                nc.vector.tensor_copy(out_sb[:, P * e : P * (e + 1)], ct_ps[:])
            nc.sync.dma_start(ctx_o[:, :], out_sb[:])

    nc.compile()
    return nc


def kernel(features, hidden_state, W1_w, W1_b, W2_w, W2_b, V_w, V_b):
    from concourse.bass_utils import run_bass_kernel_spmd

    if "nc" not in _CACHE:
        _CACHE["nc"] = _build()
    nc = _CACHE["nc"]

    features = np.ascontiguousarray(np.asarray(features, dtype=np.float32))
    hidden_state = np.ascontiguousarray(np.asarray(hidden_state, dtype=np.float32))
    W1_w = np.ascontiguousarray(np.asarray(W1_w, dtype=np.float32))
    W1_b = np.ascontiguousarray(np.asarray(W1_b, dtype=np.float32))
    W2_w = np.ascontiguousarray(np.asarray(W2_w, dtype=np.float32))
    W2_b = np.ascontiguousarray(np.asarray(W2_b, dtype=np.float32))
    V_w = np.ascontiguousarray(np.asarray(V_w, dtype=np.float32))

    in_maps = []
    for c in range(N_CORES):
        in_maps.append(
            {
                "features": np.ascontiguousarray(features[c * X : (c + 1) * X]),
                "hidden_state": np.ascontiguousarray(hidden_state[c * X : (c + 1) * X]),
                "W1_w": W1_w,
                "W1_b": W1_b,
                "W2_w": W2_w,
                "W2_b": W2_b,
                "V_w": V_w,
            }
        )

    res = run_bass_kernel_spmd(nc, in_maps, list(range(N_CORES)), **_CACHE.get("run_kwargs", {}))
    _CACHE["last_result"] = res
    alpha = np.concatenate([res.results[c]["alpha"] for c in range(N_CORES)], axis=0)
    context = np.concatenate([res.results[c]["context"] for c in range(N_CORES)], axis=0)
    return alpha, context


# revision 20
# speedup vs baseline: 1.1281x; 1.1198x over previous
"""Trainium2 Bass kernel for additive (Bahdanau) attention.

reference:
    proj_f = features @ W1_w + W1_b          # [B, L, ATT]
    proj_h = (hidden @ W2_w + W2_b)[:, None] # [B, 1, ATT]
    scores = tanh(proj_f + proj_h) @ V_w + V_b   # [B, L]
    alpha  = softmax(scores, axis=1)
    context = einsum('bl,ble->be', alpha, features)
    returns (alpha, context)

Sharding: data-parallel over batch B=64 across 8 cores (8 examples/core).
Weights replicated. No collectives.

Per-core algorithm (X = 8 examples):
  - cast-DMA features f32 -> bf16 DRAM scratch (gpsimd SWDGE).
  - HW transpose-DMA bf16 DRAM -> SBUF: fT [ENC_chunk=128, L=1024] x8.
  - main matmul in [ATT_part, L_free] orientation: lhsT = W1 chunk
    (natural layout), rhs = fT. PSUM [128, 512] f32.
  - ACT applies tanh fused with per-partition bias = (W1_b + W2_b +
    hidden @ W2_w) transposed - computed in a small f32 prepass.
  - V-dot on PE: scores[1, L] += V_chunk[128,1].T @ tanh_tile, accumulated
    over ATT chunks in PSUM.  (V_b dropped: softmax is shift-invariant.)
  - softmax per example on DVE/ACT (free-dim reduces on [1, 1024]).
  - context on DVE: tensor_tensor_reduce over fT tiles with alpha
    replicated across partitions (gpsimd partition_broadcast).
"""

import numpy as np

B, L, ENC, DEC, ATT = 64, 1024, 1024, 1024, 1024
N_CORES = 8
X = B // N_CORES  # examples per core
P = 128
NE = ENC // P  # 8
NA = ATT // P  # 8
ND = DEC // P  # 8
LH = 512       # free-dim half for fp32 PSUM bank
NL = L // LH   # 2

_CACHE = {}


def _build():
    import concourse.bacc as bacc
    import concourse.mybir as mybir
    import concourse.tile as tile

    f32, bf16 = mybir.dt.float32, mybir.dt.bfloat16
    Tanh = mybir.ActivationFunctionType.Tanh
    Exp = mybir.ActivationFunctionType.Exp
    add = mybir.AluOpType.add
    mult = mybir.AluOpType.mult
    AX = mybir.AxisListType.X

    nc = bacc.Bacc("TRN2", target_bir_lowering=False, debug=False, num_devices=N_CORES)

    feats = nc.declare_dram_parameter("features", [X, L, ENC], f32, isOutput=False)
    hid = nc.declare_dram_parameter("hidden_state", [X, DEC], f32, isOutput=False)
    w1 = nc.declare_dram_parameter("W1_w", [ENC, ATT], f32, isOutput=False)
    w1b = nc.declare_dram_parameter("W1_b", [ATT], f32, isOutput=False)
    w2 = nc.declare_dram_parameter("W2_w", [DEC, ATT], f32, isOutput=False)
    w2b = nc.declare_dram_parameter("W2_b", [ATT], f32, isOutput=False)
    vw = nc.declare_dram_parameter("V_w", [ATT], f32, isOutput=False)
    alpha_o = nc.declare_dram_parameter("alpha", [X, L], f32, isOutput=True)
    ctx_o = nc.declare_dram_parameter("context", [X, ENC], f32, isOutput=True)

    eye_dram = nc.inline_tensor(np.eye(P, dtype=np.float32), "eye128")

    with tile.TileContext(nc) as tc:
        with (
            tc.tile_pool(name="const", bufs=1) as const,
            tc.tile_pool(name="dram", bufs=4, space="DRAM") as dram,
            tc.tile_pool(name="fn", bufs=10) as fnp,
            tc.tile_pool(name="fb", bufs=6) as fbp,
            tc.tile_pool(name="ft", bufs=2 * NE + 4) as ftp,
            tc.tile_pool(name="mm", bufs=3, space="PSUM") as psum,
            tc.tile_pool(name="sc", bufs=2, space="PSUM") as spsum,
            tc.tile_pool(name="ct", bufs=1, space="PSUM") as ctpsum,
            tc.tile_pool(name="tb", bufs=6) as tp,
            tc.tile_pool(name="jk", bufs=2) as jp,
            tc.tile_pool(name="al", bufs=2) as alp,
            tc.tile_pool(name="ms", bufs=1) as ms,
        ):
            # ---------------- prep: constants & weights ----------------
            eye = const.tile([P, P], f32, tag="eye")
            nc.sync.dma_start(eye[:], eye_dram[:, :])

            w1bf = []
            for e in range(NE):
                t = const.tile([P, ATT], bf16, tag=f"w1_{e}")
                nc.gpsimd.dma_start(t[:], w1[P * e : P * (e + 1), :])
                w1bf.append(t)

            w2t = []
            for e in range(ND):
                t = const.tile([P, ATT], bf16, tag=f"w2_{e}")
                nc.gpsimd.dma_start(t[:], w2[P * e : P * (e + 1), :])
                w2t.append(t)

            # hT_all[p, c, x] = hid[x, 128c + p]  (fine-grained strided DMA)
            hT = ms.tile([P, ND, X], f32, tag="hT")
            hid_t = hid.rearrange("x (c p) -> p c x", p=P)
            for c in range(ND):
                nc.gpsimd.dma_start(hT[:, c, :], hid_t[:, c, :])
            hTb = ms.tile([P, ND, X], bf16, tag="hTb")
            nc.vector.tensor_copy(hTb[:], hT[:])

            # bias vectors transposed: bT[p, c] = v[128c + p]
            w1bT = ms.tile([P, NA], f32, tag="w1bT")
            nc.gpsimd.dma_start(w1bT[:], w1b.rearrange("(c p) -> p c", p=P))
            w2bT = ms.tile([P, NA], f32, tag="w2bT")
            nc.gpsimd.dma_start(w2bT[:], w2b.rearrange("(c p) -> p c", p=P))
            vwT = ms.tile([P, NA], f32, tag="vwT")
            nc.gpsimd.dma_start(vwT[:], vw.rearrange("(c p) -> p c", p=P))
            vwbf = ms.tile([P, NA], bf16, tag="vwbf")
            nc.vector.tensor_copy(vwbf[:], vwT[:])

            bT = ms.tile([P, NA], f32, tag="bT")
            nc.vector.tensor_add(bT[:], w1bT[:], w2bT[:])

            # proj_h transposed, plus bias: phb[p, a, x]
            phb = ms.tile([P, NA, X], f32, tag="phb")
            for a in range(NA):
                ph_ps = psum.tile([P, X], f32, tag="mm")
                for e in range(ND):
                    nc.tensor.matmul(
                        ph_ps[:],
                        w2t[e][:, P * a : P * (a + 1)],
                        hTb[:, e, :],
                        start=(e == 0),
                        stop=(e == ND - 1),
                    )
                nc.vector.tensor_scalar_add(phb[:, a, :], ph_ps[:], bT[:, a : a + 1])

            # ---------------- outputs accumulated in SBUF ----------------
            ctx_sb = ms.tile([P, NE * X], f32, tag="ctx_sb")

            # ---------------- main per-example pipeline ----------------
            # V-dot matmuls are delayed by one (a, lh) block so the PE never
            # waits on the ACT tanh of the block it just produced.
            pending = []

            def flush_pending():
                for sc_ap, vw_ap, tb_ap, st, sp in pending:
                    nc.tensor.matmul(sc_ap, vw_ap, tb_ap, start=st, stop=sp)
                pending.clear()

            def emit_cast(x):
                # f32 load (HWDGE, 16 queues) -> DVE bf16 convert -> bf16
                # store to DRAM scratch (HWDGE).  Avoids the 27 GB/s-per-queue
                # SWDGE cast path entirely.
                fbf = dram.tile([L, ENC], bf16, tag="fbf")
                for c in range(8):
                    fnat = fnp.tile([P, ENC], f32, tag="fn")
                    nc.sync.dma_start(fnat[:], feats[x, P * c : P * (c + 1), :])
                    fb = fbp.tile([P, ENC], bf16, tag="fb")
                    nc.vector.tensor_copy(fb[:], fnat[:])
                    nc.scalar.dma_start(fbf[P * c : P * (c + 1), :], fb[:])
                return fbf

            def emit_transpose(fbf):
                # HW transpose-DMA: fT[e] = fbf[:, 128e:128(e+1)].T
                fts = []
                for e in range(NE):
                    ft = ftp.tile([P, L], bf16, tag="ft")
                    nc.sync.dma_start(ft[:], fbf[:, P * e : P * (e + 1)], transpose=True)
                    fts.append(ft)
                return fts

            # prologue: stage examples 0-2
            fbfs = {0: emit_cast(0), 1: emit_cast(1)}
            ft_map = {0: emit_transpose(fbfs[0])}
            fbfs[2] = emit_cast(2)
            ft_map[1] = emit_transpose(fbfs[1])

            for x in range(X):
                if x + 3 < X:
                    fbfs[x + 3] = emit_cast(x + 3)
                if x + 2 < X:
                    ft_map[x + 2] = emit_transpose(fbfs[x + 2])
                fts = ft_map.pop(x)

                sc_ps = spsum.tile([1, L], f32, tag="sc")
                for a in range(NA):
                    for lh in range(NL):
                        pp = psum.tile([P, LH], f32, tag="mm")
                        for e in range(NE):
                            nc.tensor.matmul(
                                pp[:],
                                w1bf[e][:, P * a : P * (a + 1)],
                                fts[e][:, LH * lh : LH * (lh + 1)],
                                start=(e == 0),
                                stop=(e == NE - 1),
                            )
                        flush_pending()
                        tb = tp.tile([P, LH], bf16, tag="tb")
                        nc.scalar.activation(tb[:], pp[:], Tanh, bias=phb[:, a, x : x + 1])
                        pending.append(
                            (
                                sc_ps[:, LH * lh : LH * (lh + 1)],
                                vwbf[:, a : a + 1],
                                tb[:],
                                a == 0,
                                a == NA - 1,
                            )
                        )

                flush_pending()
                # softmax over L on partition 0
                negm = alp.tile([1, 1], f32, tag="negm")
                nc.vector.tensor_reduce(
                    negm[:], sc_ps[:], axis=AX, op=mybir.AluOpType.max, negate=True
                )
                esb = alp.tile([1, L], f32, tag="esb")
                ssum = alp.tile([1, 1], f32, tag="ssum")
                nc.scalar.activation(
                    esb[:], sc_ps[:], Exp, bias=negm[:], accum_out=ssum[:]
                )
                rinv = alp.tile([1, 1], f32, tag="rinv")
                nc.vector.reciprocal(rinv[:], ssum[:])
                a32 = alp.tile([1, L], f32, tag="a32")
                nc.vector.tensor_scalar_mul(a32[:], esb[:], rinv[:])
                nc.sync.dma_start(alpha_o[x, :], a32[:])
                abf = alp.tile([1, L], bf16, tag="abf")
                nc.vector.tensor_scalar_mul(abf[:], esb[:], rinv[:])
                arep = alp.tile([P, L], bf16, tag="arep")
                nc.gpsimd.partition_broadcast(arep[:], abf[:])

                # context: ctx[e-chunk] = sum_l fT[e][:, l] * alpha[l]
                for e in range(NE):
                    jk = jp.tile([P, L], f32, tag="jk")
                    nc.vector.scalar_tensor_tensor(
                        out=jk[:],
                        in0=fts[e][:],
                        scalar=1.0,
                        in1=arep[:],
                        op0=mult,
                        op1=mult,
                        accum_out=ctx_sb[:, X * e + x : X * e + x + 1],
                    )

            # ---------------- epilogue: outputs ----------------
            out_sb = ms.tile([X, ENC], f32, tag="out_sb")
            for e in range(NE):
                ct_ps = ctpsum.tile([X, P], f32, tag="ctps")
                nc.tensor.transpose(ct_ps[:], ctx_sb[:, X * e : X * (e + 1)], eye[:])
                nc.vector.tensor_copy(out_sb[:, P * e : P * (e + 1)], ct_ps[:])
            nc.sync.dma_start(ctx_o[:, :], out_sb[:])

    nc.compile()
    return nc


def kernel(features, hidden_state, W1_w, W1_b, W2_w, W2_b, V_w, V_b):
    from concourse.bass_utils import run_bass_kernel_spmd

    if "nc" not in _CACHE:
        _CACHE["nc"] = _build()
    nc = _CACHE["nc"]

    features = np.ascontiguousarray(np.asarray(features, dtype=np.float32))
    hidden_state = np.ascontiguousarray(np.asarray(hidden_state, dtype=np.float32))
    W1_w = np.ascontiguousarray(np.asarray(W1_w, dtype=np.float32))
    W1_b = np.ascontiguousarray(np.asarray(W1_b, dtype=np.float32))
    W2_w = np.ascontiguousarray(np.asarray(W2_w, dtype=np.float32))
    W2_b = np.ascontiguousarray(np.asarray(W2_b, dtype=np.float32))
    V_w = np.ascontiguousarray(np.asarray(V_w, dtype=np.float32))

    in_maps = []
    for c in range(N_CORES):
        in_maps.append(
            {
                "features": np.ascontiguousarray(features[c * X : (c + 1) * X]),
                "hidden_state": np.ascontiguousarray(hidden_state[c * X : (c + 1) * X]),
                "W1_w": W1_w,
                "W1_b": W1_b,
                "W2_w": W2_w,
                "W2_b": W2_b,
                "V_w": V_w,
            }
        )

    res = run_bass_kernel_spmd(nc, in_maps, list(range(N_CORES)), **_CACHE.get("run_kwargs", {}))
    _CACHE["last_result"] = res
    alpha = np.concatenate([res.results[c]["alpha"] for c in range(N_CORES)], axis=0)
    context = np.concatenate([res.results[c]["context"] for c in range(N_CORES)], axis=0)
    return alpha, context


# revision 23
# speedup vs baseline: 1.4623x; 1.2963x over previous
"""Trainium2 Bass kernel for additive (Bahdanau) attention.

reference:
    proj_f = features @ W1_w + W1_b          # [B, L, ATT]
    proj_h = (hidden @ W2_w + W2_b)[:, None] # [B, 1, ATT]
    scores = tanh(proj_f + proj_h) @ V_w + V_b   # [B, L]
    alpha  = softmax(scores, axis=1)
    context = einsum('bl,ble->be', alpha, features)
    returns (alpha, context)

Sharding: data-parallel over batch B=64 across 8 cores (8 examples/core).
Weights replicated. No collectives.

Per-core algorithm (X = 8 examples):
  - f32 feature loads over the 16 HWDGE queues (the only bulk HBM traffic,
    32 MB/core), DVE-convert to bf16.
  - features transposed ON-CHIP: PE transpose-mode 128x128 blocks, 4 blocks
    per PSUM bank, one strided DVE copy per bank into a per-example
    fT [128, ENCxL] tile.  (DMA-transpose via a DRAM bf16 bounce was tried
    and is queue-descriptor-bound: 71 MB through ~20 GB/s/queue.)
  - main matmul in [ATT_part, L_free] orientation: lhsT = W1 chunk
    (natural layout, bf16), rhs = fT slices.  PSUM [128, 512] f32.
  - ACT applies tanh fused with per-partition bias = (W1_b + W2_b +
    hidden @ W2_w) transposed - computed in a small prepass.
  - V-dot on PE: scores[1, 512] += V_chunk[128,1].T @ tanh_tile, accumulated
    over ATT chunks in PSUM; V-dot MMs trail the tanh by one block so the
    PE never waits on ACT.  (V_b dropped: softmax is shift-invariant.)
  - softmax per example on DVE/ACT (free-dim reduces on [1, 1024]).
  - context on DVE: scalar_tensor_tensor over fT with alpha replicated
    across partitions (gpsimd partition_broadcast).
"""

import numpy as np

B, L, ENC, DEC, ATT = 64, 1024, 1024, 1024, 1024
N_CORES = 8
X = B // N_CORES  # examples per core
P = 128
NE = ENC // P  # 8
NA = ATT // P  # 8
ND = DEC // P  # 8
LH = 512       # free-dim half for fp32 PSUM bank
NL = L // LH   # 2

_CACHE = {}


def _build():
    import concourse.bacc as bacc
    import concourse.mybir as mybir
    import concourse.tile as tile

    f32, bf16 = mybir.dt.float32, mybir.dt.bfloat16
    Tanh = mybir.ActivationFunctionType.Tanh
    Exp = mybir.ActivationFunctionType.Exp
    mult = mybir.AluOpType.mult
    AX = mybir.AxisListType.X

    nc = bacc.Bacc("TRN2", target_bir_lowering=False, debug=False, num_devices=N_CORES)

    feats = nc.declare_dram_parameter("features", [X, L, ENC], f32, isOutput=False)
    hid = nc.declare_dram_parameter("hidden_state", [X, DEC], f32, isOutput=False)
    w1 = nc.declare_dram_parameter("W1_w", [ENC, ATT], f32, isOutput=False)
    w1b = nc.declare_dram_parameter("W1_b", [ATT], f32, isOutput=False)
    w2 = nc.declare_dram_parameter("W2_w", [DEC, ATT], f32, isOutput=False)
    w2b = nc.declare_dram_parameter("W2_b", [ATT], f32, isOutput=False)
    vw = nc.declare_dram_parameter("V_w", [ATT], f32, isOutput=False)
    alpha_o = nc.declare_dram_parameter("alpha", [X, L], f32, isOutput=True)
    ctx_o = nc.declare_dram_parameter("context", [X, ENC], f32, isOutput=True)

    eye_dram = nc.inline_tensor(np.eye(P, dtype=np.float32), "eye128")

    with tile.TileContext(nc) as tc:
        with (
            tc.tile_pool(name="const", bufs=1) as const,
            tc.tile_pool(name="fn", bufs=6) as fnp,
            tc.tile_pool(name="fb", bufs=16) as fbp,
            tc.tile_pool(name="ft", bufs=3) as ftp,
            tc.tile_pool(name="mm", bufs=3, space="PSUM") as psum,
            tc.tile_pool(name="sc", bufs=3, space="PSUM") as spsum,
            tc.tile_pool(name="tp", bufs=2, space="PSUM") as tpsum,
            tc.tile_pool(name="tb", bufs=6) as tp,
            tc.tile_pool(name="jk", bufs=2) as jp,
            tc.tile_pool(name="al", bufs=2) as alp,
            tc.tile_pool(name="ms", bufs=1) as ms,
        ):
            # ---------------- prep: constants & weights ----------------
            eye = const.tile([P, P], f32, tag="eye")
            nc.sync.dma_start(eye[:], eye_dram[:, :])
            eye_bf = const.tile([P, P], bf16, tag="eye_bf")
            nc.vector.tensor_copy(eye_bf[:], eye[:])

            w1bf = []
            for e in range(NE):
                t = const.tile([P, ATT], bf16, tag=f"w1_{e}")
                nc.gpsimd.dma_start(t[:], w1[P * e : P * (e + 1), :])
                w1bf.append(t)

            w2t = []
            for e in range(ND):
                t = const.tile([P, ATT], bf16, tag=f"w2_{e}")
                nc.gpsimd.dma_start(t[:], w2[P * e : P * (e + 1), :])
                w2t.append(t)

            # hT_all[p, c, x] = hid[x, 128c + p]  (fine-grained strided DMA)
            hT = ms.tile([P, ND, X], f32, tag="hT")
            hid_t = hid.rearrange("x (c p) -> p c x", p=P)
            for c in range(ND):
                nc.gpsimd.dma_start(hT[:, c, :], hid_t[:, c, :])
            hTb = ms.tile([P, ND, X], bf16, tag="hTb")
            nc.vector.tensor_copy(hTb[:], hT[:])

            # bias vectors transposed: bT[p, c] = v[128c + p]
            w1bT = ms.tile([P, NA], f32, tag="w1bT")
            nc.gpsimd.dma_start(w1bT[:], w1b.rearrange("(c p) -> p c", p=P))
            w2bT = ms.tile([P, NA], f32, tag="w2bT")
            nc.gpsimd.dma_start(w2bT[:], w2b.rearrange("(c p) -> p c", p=P))
            vwT = ms.tile([P, NA], f32, tag="vwT")
            nc.gpsimd.dma_start(vwT[:], vw.rearrange("(c p) -> p c", p=P))
            vwbf = ms.tile([P, NA], bf16, tag="vwbf")
            nc.vector.tensor_copy(vwbf[:], vwT[:])

            bT = ms.tile([P, NA], f32, tag="bT")
            nc.vector.tensor_add(bT[:], w1bT[:], w2bT[:])

            # proj_h transposed, plus bias: phb[p, a, x]
            phb = ms.tile([P, NA, X], f32, tag="phb")
            for a in range(NA):
                ph_ps = psum.tile([P, X], f32, tag="mm")
                for e in range(ND):
                    nc.tensor.matmul(
                        ph_ps[:],
                        w2t[e][:, P * a : P * (a + 1)],
                        hTb[:, e, :],
                        start=(e == 0),
                        stop=(e == ND - 1),
                    )
                nc.vector.tensor_scalar_add(phb[:, a, :], ph_ps[:], bT[:, a : a + 1])

            ctx_sb = ms.tile([P, NE * X], f32, tag="ctx_sb")

            # ---------------- per-example staging ----------------
            def emit_load_convert(x):
                # f32 loads split 4-ways per l-chunk to spread queues,
                # then DVE bf16 convert.
                fbs = []
                for c in range(8):
                    fnat = fnp.tile([P, ENC], f32, tag="fn")
                    for q in range(4):
                        nc.sync.dma_start(
                            fnat[:, 256 * q : 256 * (q + 1)],
                            feats[x, P * c : P * (c + 1), 256 * q : 256 * (q + 1)],
                        )
                    fb = fbp.tile([P, ENC], bf16, tag="fb")
                    nc.vector.tensor_copy(fb[:], fnat[:])
                    fbs.append(fb)
                return fbs

            def alloc_ft():
                return ms_ft()

            def ms_ft():
                t = ftp.tile([P, NE * L], bf16, tag="ft")
                return t

            def ft_view(ft):
                return ft.rearrange("p (e lc c) -> p e lc c", e=NE, lc=NE)

            def emit_transpose_group(fbs, ft, lc, h):
                # transpose blocks (e in [4h, 4h+4), l-chunk lc) of the
                # natural bf16 tiles into ft columns e*L + 128*lc.
                tps = tpsum.tile([P, 4 * P], bf16, tag="tp")
                for j in range(4):
                    e = 4 * h + j
                    nc.tensor.transpose(
                        tps[:, P * j : P * (j + 1)],
                        fbs[lc][:, P * e : P * (e + 1)],
                        eye_bf[:],
                    )
                dst = ft_view(ft)[:, 4 * h : 4 * h + 4, lc, :]
                src = tps.rearrange("p (e c) -> p e c", e=4)
                nc.vector.tensor_copy(dst, src)

            # ---------------- main per-example pipeline ----------------
            # V-dot matmuls trail the tanh by one block so the PE never
            # waits on ACT.
            pending = []

            def flush_pending():
                for sc_ap, vw_ap, tb_ap, st, sp in pending:
                    nc.tensor.matmul(sc_ap, vw_ap, tb_ap, start=st, stop=sp)
                pending.clear()

            # prologue: stage examples 0 and 1
            fb_map = {0: emit_load_convert(0)}
            ft_map = {0: alloc_ft()}
            for lc in range(NE):
                for h in range(2):
                    emit_transpose_group(fb_map[0], ft_map[0], lc, h)
            fb_map[1] = emit_load_convert(1)

            for x in range(X):
                if x + 2 < X:
                    fb_map[x + 2] = emit_load_convert(x + 2)
                if x + 1 < X:
                    ft_map[x + 1] = alloc_ft()
                    tgroups = [(lc, h) for lc in range(NE) for h in range(2)]
                else:
                    tgroups = []
                ft = ft_map[x]
                ftv = ft_view(ft)

                sc_h = {}
                mb = 0  # micro-block index
                for a in range(NA):
                    for lh in range(NL):
                        pp = psum.tile([P, LH], f32, tag="mm")
                        for e in range(NE):
                            nc.tensor.matmul(
                                pp[:],
                                w1bf[e][:, P * a : P * (a + 1)],
                                ft[:, e * L + LH * lh : e * L + LH * (lh + 1)],
                                start=(e == 0),
                                stop=(e == NE - 1),
                            )
                        flush_pending()
                        # stage next example's transposes between MM blocks
                        if tgroups and mb >= 2:
                            lc, h = tgroups.pop(0)
                            emit_transpose_group(fb_map[x + 1], ft_map[x + 1], lc, h)
                        mb += 1
                        if lh not in sc_h:
                            sc_h[lh] = spsum.tile([1, LH], f32, tag="sc", name=f"sch{lh}")
                        tb = tp.tile([P, LH], bf16, tag="tb")
                        nc.scalar.activation(tb[:], pp[:], Tanh, bias=phb[:, a, x : x + 1])
                        pending.append(
                            (sc_h[lh][:], vwbf[:, a : a + 1], tb[:], a == 0, a == NA - 1)
                        )

                flush_pending()
                for lc, h in tgroups:
                    emit_transpose_group(fb_map[x + 1], ft_map[x + 1], lc, h)
                fb_map.pop(x, None)

                # softmax over L on partition 0
                scores = alp.tile([1, L], f32, tag="scores")
                nc.vector.tensor_copy(scores[:, 0:LH], sc_h[0][:])
                nc.vector.tensor_copy(scores[:, LH:L], sc_h[1][:])
                negm = alp.tile([1, 1], f32, tag="negm")
                nc.vector.tensor_reduce(
                    negm[:], scores[:], axis=AX, op=mybir.AluOpType.max, negate=True
                )
                esb = alp.tile([1, L], f32, tag="esb")
                ssum = alp.tile([1, 1], f32, tag="ssum")
                nc.scalar.activation(esb[:], scores[:], Exp, bias=negm[:], accum_out=ssum[:])
                rinv = alp.tile([1, 1], f32, tag="rinv")
                nc.vector.reciprocal(rinv[:], ssum[:])
                a32 = alp.tile([1, L], f32, tag="a32")
                nc.vector.tensor_scalar_mul(a32[:], esb[:], rinv[:])
                nc.sync.dma_start(alpha_o[x, :], a32[:])
                abf = alp.tile([1, L], bf16, tag="abf")
                nc.vector.tensor_scalar_mul(abf[:], esb[:], rinv[:])
                arep = alp.tile([P, L], bf16, tag="arep")
                nc.gpsimd.partition_broadcast(arep[:], abf[:])

                # context: ctx[e-chunk] = sum_l fT[e][:, l] * alpha[l]
                for e in range(NE):
                    jk = jp.tile([P, L], f32, tag="jk")
                    nc.vector.scalar_tensor_tensor(
                        out=jk[:],
                        in0=ft[:, e * L : (e + 1) * L],
                        scalar=1.0,
                        in1=arep[:],
                        op0=mult,
                        op1=mult,
                        accum_out=ctx_sb[:, X * e + x : X * e + x + 1],
                    )

            # ---------------- epilogue: outputs ----------------
            out_sb = ms.tile([X, ENC], f32, tag="out_sb")
            for e in range(NE):
                ct_ps = tpsum.tile([X, P], f32, tag="tp")
                nc.tensor.transpose(ct_ps[:], ctx_sb[:, X * e : X * (e + 1)], eye[:])
                nc.vector.tensor_copy(out_sb[:, P * e : P * (e + 1)], ct_ps[:])
            nc.sync.dma_start(ctx_o[:, :], out_sb[:])

    nc.compile()
    return nc


def kernel(features, hidden_state, W1_w, W1_b, W2_w, W2_b, V_w, V_b):
    from concourse.bass_utils import run_bass_kernel_spmd

    if "nc" not in _CACHE:
        _CACHE["nc"] = _build()
    nc = _CACHE["nc"]

    features = np.ascontiguousarray(np.asarray(features, dtype=np.float32))
    hidden_state = np.ascontiguousarray(np.asarray(hidden_state, dtype=np.float32))
    W1_w = np.ascontiguousarray(np.asarray(W1_w, dtype=np.float32))
    W1_b = np.ascontiguousarray(np.asarray(W1_b, dtype=np.float32))
    W2_w = np.ascontiguousarray(np.asarray(W2_w, dtype=np.float32))
    W2_b = np.ascontiguousarray(np.asarray(W2_b, dtype=np.float32))
    V_w = np.ascontiguousarray(np.asarray(V_w, dtype=np.float32))

    in_maps = []
    for c in range(N_CORES):
        in_maps.append(
            {
                "features": np.ascontiguousarray(features[c * X : (c + 1) * X]),
                "hidden_state": np.ascontiguousarray(hidden_state[c * X : (c + 1) * X]),
                "W1_w": W1_w,
                "W1_b": W1_b,
                "W2_w": W2_w,
                "W2_b": W2_b,
                "V_w": V_w,
            }
        )

    res = run_bass_kernel_spmd(nc, in_maps, list(range(N_CORES)), **_CACHE.get("run_kwargs", {}))
    _CACHE["last_result"] = res
    alpha = np.concatenate([res.results[c]["alpha"] for c in range(N_CORES)], axis=0)
    context = np.concatenate([res.results[c]["context"] for c in range(N_CORES)], axis=0)
    return alpha, context


# revision 25
# speedup vs baseline: 1.4753x; 1.0089x over previous
"""Trainium2 Bass kernel for additive (Bahdanau) attention.

reference:
    proj_f = features @ W1_w + W1_b          # [B, L, ATT]
    proj_h = (hidden @ W2_w + W2_b)[:, None] # [B, 1, ATT]
    scores = tanh(proj_f + proj_h) @ V_w + V_b   # [B, L]
    alpha  = softmax(scores, axis=1)
    context = einsum('bl,ble->be', alpha, features)
    returns (alpha, context)

Sharding: data-parallel over batch B=64 across 8 cores (8 examples/core).
Weights replicated. No collectives.

Per-core algorithm (X = 8 examples):
  - f32 feature loads over the 16 HWDGE queues (the only bulk HBM traffic,
    32 MB/core), DVE-convert to bf16.
  - features transposed ON-CHIP: PE transpose-mode 128x128 blocks, 4 blocks
    per PSUM bank, one strided DVE copy per bank into a per-example
    fT [128, ENCxL] tile.  (DMA-transpose via a DRAM bf16 bounce was tried
    and is queue-descriptor-bound: 71 MB through ~20 GB/s/queue.)
  - main matmul in [ATT_part, L_free] orientation: lhsT = W1 chunk
    (natural layout, bf16), rhs = fT slices.  PSUM [128, 512] f32.
  - ACT applies tanh fused with per-partition bias = (W1_b + W2_b +
    hidden @ W2_w) transposed - computed in a small prepass.
  - V-dot on PE: scores[1, 512] += V_chunk[128,1].T @ tanh_tile, accumulated
    over ATT chunks in PSUM; V-dot MMs trail the tanh by one block so the
    PE never waits on ACT.  (V_b dropped: softmax is shift-invariant.)
  - softmax per example on DVE/ACT (free-dim reduces on [1, 1024]).
  - context on DVE: scalar_tensor_tensor over fT with alpha replicated
    across partitions (gpsimd partition_broadcast).
"""

import numpy as np

B, L, ENC, DEC, ATT = 64, 1024, 1024, 1024, 1024
N_CORES = 8
X = B // N_CORES  # examples per core
P = 128
NE = ENC // P  # 8
NA = ATT // P  # 8
ND = DEC // P  # 8
LH = 512       # free-dim half for fp32 PSUM bank
NL = L // LH   # 2

_CACHE = {}


def _build():
    import concourse.bacc as bacc
    import concourse.mybir as mybir
    import concourse.tile as tile

    f32, bf16 = mybir.dt.float32, mybir.dt.bfloat16
    Tanh = mybir.ActivationFunctionType.Tanh
    Exp = mybir.ActivationFunctionType.Exp
    mult = mybir.AluOpType.mult
    AX = mybir.AxisListType.X

    nc = bacc.Bacc("TRN2", target_bir_lowering=False, debug=False, num_devices=N_CORES)

    feats = nc.declare_dram_parameter("features", [X, L, ENC], f32, isOutput=False)
    hid = nc.declare_dram_parameter("hidden_state", [X, DEC], f32, isOutput=False)
    w1 = nc.declare_dram_parameter("W1_w", [ENC, ATT], f32, isOutput=False)
    w1b = nc.declare_dram_parameter("W1_b", [ATT], f32, isOutput=False)
    w2 = nc.declare_dram_parameter("W2_w", [DEC, ATT], f32, isOutput=False)
    w2b = nc.declare_dram_parameter("W2_b", [ATT], f32, isOutput=False)
    vw = nc.declare_dram_parameter("V_w", [ATT], f32, isOutput=False)
    alpha_o = nc.declare_dram_parameter("alpha", [X, L], f32, isOutput=True)
    ctx_o = nc.declare_dram_parameter("context", [X, ENC], f32, isOutput=True)

    eye_dram = nc.inline_tensor(np.eye(P, dtype=np.float32), "eye128")

    with tile.TileContext(nc) as tc:
        with (
            tc.tile_pool(name="const", bufs=1) as const,
            tc.tile_pool(name="fn", bufs=6) as fnp,
            tc.tile_pool(name="fb", bufs=16) as fbp,
            tc.tile_pool(name="ft", bufs=3) as ftp,
            tc.tile_pool(name="mm", bufs=3, space="PSUM") as psum,
            tc.tile_pool(name="sc", bufs=3, space="PSUM") as spsum,
            tc.tile_pool(name="tp", bufs=2, space="PSUM") as tpsum,
            tc.tile_pool(name="tb", bufs=6) as tp,
            tc.tile_pool(name="jk", bufs=2) as jp,
            tc.tile_pool(name="al", bufs=2) as alp,
            tc.tile_pool(name="ms", bufs=1) as ms,
        ):
            # ---------------- prep: constants & weights ----------------
            eye = const.tile([P, P], f32, tag="eye")
            nc.sync.dma_start(eye[:], eye_dram[:, :])
            eye_bf = const.tile([P, P], bf16, tag="eye_bf")
            nc.vector.tensor_copy(eye_bf[:], eye[:])

            w1bf = []
            for e in range(NE):
                t = const.tile([P, ATT], bf16, tag=f"w1_{e}")
                nc.gpsimd.dma_start(t[:], w1[P * e : P * (e + 1), :])
                w1bf.append(t)

            w2t = []
            for e in range(ND):
                t = const.tile([P, ATT], bf16, tag=f"w2_{e}")
                nc.gpsimd.dma_start(t[:], w2[P * e : P * (e + 1), :])
                w2t.append(t)

            # hT_all[p, c, x] = hid[x, 128c + p]  (fine-grained strided DMA)
            hT = ms.tile([P, ND, X], f32, tag="hT")
            hid_t = hid.rearrange("x (c p) -> p c x", p=P)
            for c in range(ND):
                nc.gpsimd.dma_start(hT[:, c, :], hid_t[:, c, :])
            hTb = ms.tile([P, ND, X], bf16, tag="hTb")
            nc.vector.tensor_copy(hTb[:], hT[:])

            # bias vectors transposed: bT[p, c] = v[128c + p]
            w1bT = ms.tile([P, NA], f32, tag="w1bT")
            nc.gpsimd.dma_start(w1bT[:], w1b.rearrange("(c p) -> p c", p=P))
            w2bT = ms.tile([P, NA], f32, tag="w2bT")
            nc.gpsimd.dma_start(w2bT[:], w2b.rearrange("(c p) -> p c", p=P))
            vwT = ms.tile([P, NA], f32, tag="vwT")
            nc.gpsimd.dma_start(vwT[:], vw.rearrange("(c p) -> p c", p=P))
            vwbf = ms.tile([P, NA], bf16, tag="vwbf")
            nc.vector.tensor_copy(vwbf[:], vwT[:])

            bT = ms.tile([P, NA], f32, tag="bT")
            nc.vector.tensor_add(bT[:], w1bT[:], w2bT[:])

            # proj_h transposed, plus bias: phb[p, a, x]
            phb = ms.tile([P, NA, X], f32, tag="phb")
            for a in range(NA):
                ph_ps = psum.tile([P, X], f32, tag="mm")
                for e in range(ND):
                    nc.tensor.matmul(
                        ph_ps[:],
                        w2t[e][:, P * a : P * (a + 1)],
                        hTb[:, e, :],
                        start=(e == 0),
                        stop=(e == ND - 1),
                    )
                nc.vector.tensor_scalar_add(phb[:, a, :], ph_ps[:], bT[:, a : a + 1])


            # ---------------- per-example staging ----------------
            def emit_load_convert(x):
                # f32 loads split 4-ways per l-chunk to spread queues,
                # then DVE bf16 convert.
                fbs = []
                for c in range(8):
                    fnat = fnp.tile([P, ENC], f32, tag="fn")
                    for q in range(4):
                        nc.sync.dma_start(
                            fnat[:, 256 * q : 256 * (q + 1)],
                            feats[x, P * c : P * (c + 1), 256 * q : 256 * (q + 1)],
                        )
                    fb = fbp.tile([P, ENC], bf16, tag="fb")
                    nc.vector.tensor_copy(fb[:], fnat[:])
                    fbs.append(fb)
                return fbs

            def alloc_ft():
                return ms_ft()

            def ms_ft():
                t = ftp.tile([P, NE * L], bf16, tag="ft")
                return t

            def ft_view(ft):
                return ft.rearrange("p (e lc c) -> p e lc c", e=NE, lc=NE)

            def emit_transpose_group(fbs, ft, lc, h):
                # transpose blocks (e in [4h, 4h+4), l-chunk lc) of the
                # natural bf16 tiles into ft columns e*L + 128*lc.
                tps = tpsum.tile([P, 4 * P], bf16, tag="tp")
                for j in range(4):
                    e = 4 * h + j
                    nc.tensor.transpose(
                        tps[:, P * j : P * (j + 1)],
                        fbs[lc][:, P * e : P * (e + 1)],
                        eye_bf[:],
                    )
                dst = ft_view(ft)[:, 4 * h : 4 * h + 4, lc, :]
                src = tps.rearrange("p (e c) -> p e c", e=4)
                nc.vector.tensor_copy(dst, src)

            # ---------------- main per-example pipeline ----------------
            # V-dot matmuls trail the tanh by one block so the PE never
            # waits on ACT.
            pending = []

            def flush_pending():
                for sc_ap, vw_ap, tb_ap, st, sp in pending:
                    nc.tensor.matmul(sc_ap, vw_ap, tb_ap, start=st, stop=sp)
                pending.clear()

            # prologue: stage examples 0 and 1
            fb_map = {0: emit_load_convert(0)}
            ft_map = {0: alloc_ft()}
            for lc in range(NE):
                for h in range(2):
                    emit_transpose_group(fb_map[0], ft_map[0], lc, h)
            fb_map[1] = emit_load_convert(1)

            for x in range(X):
                if x + 2 < X:
                    fb_map[x + 2] = emit_load_convert(x + 2)
                if x + 1 < X:
                    ft_map[x + 1] = alloc_ft()
                    tgroups = [(lc, h) for lc in range(NE) for h in range(2)]
                else:
                    tgroups = []
                ft = ft_map[x]
                ftv = ft_view(ft)

                sc_h = {}
                mb = 0  # micro-block index
                for a in range(NA):
                    for lh in range(NL):
                        pp = psum.tile([P, LH], f32, tag="mm")
                        for e in range(NE):
                            nc.tensor.matmul(
                                pp[:],
                                w1bf[e][:, P * a : P * (a + 1)],
                                ft[:, e * L + LH * lh : e * L + LH * (lh + 1)],
                                start=(e == 0),
                                stop=(e == NE - 1),
                            )
                        flush_pending()
                        # stage next example's transposes between MM blocks
                        if tgroups and mb >= 2:
                            lc, h = tgroups.pop(0)
                            emit_transpose_group(fb_map[x + 1], ft_map[x + 1], lc, h)
                        mb += 1
                        if lh not in sc_h:
                            sc_h[lh] = spsum.tile([1, LH], f32, tag="sc", name=f"sch{lh}")
                        tb = tp.tile([P, LH], bf16, tag="tb")
                        nc.scalar.activation(tb[:], pp[:], Tanh, bias=phb[:, a, x : x + 1])
                        pending.append(
                            (sc_h[lh][:], vwbf[:, a : a + 1], tb[:], a == 0, a == NA - 1)
                        )

                flush_pending()
                for lc, h in tgroups:
                    emit_transpose_group(fb_map[x + 1], ft_map[x + 1], lc, h)
                fb_map.pop(x, None)

                # softmax over L on partition 0
                scores = alp.tile([1, L], f32, tag="scores")
                nc.vector.tensor_copy(scores[:, 0:LH], sc_h[0][:])
                nc.vector.tensor_copy(scores[:, LH:L], sc_h[1][:])
                negm = alp.tile([1, 1], f32, tag="negm")
                nc.vector.tensor_reduce(
                    negm[:], scores[:], axis=AX, op=mybir.AluOpType.max, negate=True
                )
                esb = alp.tile([1, L], f32, tag="esb")
                ssum = alp.tile([1, 1], f32, tag="ssum")
                nc.scalar.activation(esb[:], scores[:], Exp, bias=negm[:], accum_out=ssum[:])
                rinv = alp.tile([1, 1], f32, tag="rinv")
                nc.vector.reciprocal(rinv[:], ssum[:])
                a32 = alp.tile([1, L], f32, tag="a32")
                nc.vector.tensor_scalar_mul(a32[:], esb[:], rinv[:])
                nc.sync.dma_start(alpha_o[x, :], a32[:])
                abf = alp.tile([1, L], bf16, tag="abf")
                nc.vector.tensor_scalar_mul(abf[:], esb[:], rinv[:])
                arep = alp.tile([P, L], bf16, tag="arep")
                nc.gpsimd.partition_broadcast(arep[:], abf[:])

                # context: ctx[e-chunk] = sum_l fT[e][:, l] * alpha[l]
                ctx_x = alp.tile([P, NE], f32, tag="ctx_x")
                for e in range(NE):
                    jk = jp.tile([P, L], f32, tag="jk")
                    nc.vector.scalar_tensor_tensor(
                        out=jk[:],
                        in0=ft[:, e * L : (e + 1) * L],
                        scalar=1.0,
                        in1=arep[:],
                        op0=mult,
                        op1=mult,
                        accum_out=ctx_x[:, e : e + 1],
                    )
                # transpose [128, 8] -> [8, 128] and ship context[x] out
                ct_ps = tpsum.tile([X, P], f32, tag="tp", name=f"ctps{x}")
                nc.tensor.transpose(ct_ps[:], ctx_x[:], eye[:])
                ctr = alp.tile([X, P], f32, tag="ctr")
                nc.vector.tensor_copy(ctr[:], ct_ps[:])
                nc.sync.dma_start(ctx_o.rearrange("x (e c) -> x e c", e=NE)[x], ctr[:])


    nc.compile()
    return nc


def kernel(features, hidden_state, W1_w, W1_b, W2_w, W2_b, V_w, V_b):
    from concourse.bass_utils import run_bass_kernel_spmd

    if "nc" not in _CACHE:
        _CACHE["nc"] = _build()
    nc = _CACHE["nc"]

    features = np.ascontiguousarray(np.asarray(features, dtype=np.float32))
    hidden_state = np.ascontiguousarray(np.asarray(hidden_state, dtype=np.float32))
    W1_w = np.ascontiguousarray(np.asarray(W1_w, dtype=np.float32))
    W1_b = np.ascontiguousarray(np.asarray(W1_b, dtype=np.float32))
    W2_w = np.ascontiguousarray(np.asarray(W2_w, dtype=np.float32))
    W2_b = np.ascontiguousarray(np.asarray(W2_b, dtype=np.float32))
    V_w = np.ascontiguousarray(np.asarray(V_w, dtype=np.float32))

    in_maps = []
    for c in range(N_CORES):
        in_maps.append(
            {
                "features": np.ascontiguousarray(features[c * X : (c + 1) * X]),
                "hidden_state": np.ascontiguousarray(hidden_state[c * X : (c + 1) * X]),
                "W1_w": W1_w,
                "W1_b": W1_b,
                "W2_w": W2_w,
                "W2_b": W2_b,
                "V_w": V_w,
            }
        )

    res = run_bass_kernel_spmd(nc, in_maps, list(range(N_CORES)), **_CACHE.get("run_kwargs", {}))
    _CACHE["last_result"] = res
    alpha = np.concatenate([res.results[c]["alpha"] for c in range(N_CORES)], axis=0)
    context = np.concatenate([res.results[c]["context"] for c in range(N_CORES)], axis=0)
    return alpha, context


# revision 30
# speedup vs baseline: 1.4756x; 1.0002x over previous
"""Trainium2 Bass kernel for additive (Bahdanau) attention.

reference:
    proj_f = features @ W1_w + W1_b          # [B, L, ATT]
    proj_h = (hidden @ W2_w + W2_b)[:, None] # [B, 1, ATT]
    scores = tanh(proj_f + proj_h) @ V_w + V_b   # [B, L]
    alpha  = softmax(scores, axis=1)
    context = einsum('bl,ble->be', alpha, features)
    returns (alpha, context)

Sharding: data-parallel over batch B=64 across 8 cores (8 examples/core).
Weights replicated. No collectives.

Per-core algorithm (X = 8 examples):
  - f32 feature loads over the 16 HWDGE queues (the only bulk HBM traffic,
    32 MB/core), DVE-convert to bf16.
  - features transposed ON-CHIP: PE transpose-mode 128x128 blocks, 4 blocks
    per PSUM bank, one strided DVE copy per bank into a per-example
    fT [128, ENCxL] tile.  (DMA-transpose via a DRAM bf16 bounce was tried
    and is queue-descriptor-bound: 71 MB through ~20 GB/s/queue.)
  - main matmul in [ATT_part, L_free] orientation: lhsT = W1 chunk
    (natural layout, bf16), rhs = fT slices.  PSUM [128, 512] f32.
  - ACT applies tanh fused with per-partition bias = (W1_b + W2_b +
    hidden @ W2_w) transposed - computed in a small prepass.
  - V-dot on PE: scores[1, 512] += V_chunk[128,1].T @ tanh_tile, accumulated
    over ATT chunks in PSUM; V-dot MMs trail the tanh by one block so the
    PE never waits on ACT.  (V_b dropped: softmax is shift-invariant.)
  - softmax per example on DVE/ACT (free-dim reduces on [1, 1024]).
  - context on DVE: scalar_tensor_tensor over fT with alpha replicated
    across partitions (gpsimd partition_broadcast).
"""

import numpy as np

B, L, ENC, DEC, ATT = 64, 1024, 1024, 1024, 1024
N_CORES = 8
X = B // N_CORES  # examples per core
P = 128
NE = ENC // P  # 8
NA = ATT // P  # 8
ND = DEC // P  # 8
LH = 512       # free-dim half for fp32 PSUM bank
NL = L // LH   # 2

_CACHE = {}


def _build():
    import concourse.bacc as bacc
    import concourse.mybir as mybir
    import concourse.tile as tile

    f32, bf16 = mybir.dt.float32, mybir.dt.bfloat16
    Tanh = mybir.ActivationFunctionType.Tanh
    Exp = mybir.ActivationFunctionType.Exp
    mult = mybir.AluOpType.mult
    AX = mybir.AxisListType.X

    nc = bacc.Bacc("TRN2", target_bir_lowering=False, debug=False, num_devices=N_CORES)

    feats = nc.declare_dram_parameter("features", [X, L, ENC], f32, isOutput=False)
    hid = nc.declare_dram_parameter("hidden_state", [X, DEC], f32, isOutput=False)
    w1 = nc.declare_dram_parameter("W1_w", [ENC, ATT], f32, isOutput=False)
    w1b = nc.declare_dram_parameter("W1_b", [ATT], f32, isOutput=False)
    w2 = nc.declare_dram_parameter("W2_w", [DEC, ATT], f32, isOutput=False)
    w2b = nc.declare_dram_parameter("W2_b", [ATT], f32, isOutput=False)
    vw = nc.declare_dram_parameter("V_w", [ATT], f32, isOutput=False)
    alpha_o = nc.declare_dram_parameter("alpha", [X, L], f32, isOutput=True)
    ctx_o = nc.declare_dram_parameter("context", [X, ENC], f32, isOutput=True)

    eye_dram = nc.inline_tensor(np.eye(P, dtype=np.float32), "eye128")

    with tile.TileContext(nc) as tc:
        with (
            tc.tile_pool(name="const", bufs=1) as const,
            tc.tile_pool(name="fn", bufs=5) as fnp,
            tc.tile_pool(name="fb", bufs=12) as fbp,
            tc.tile_pool(name="ft", bufs=3) as ftp,
            tc.tile_pool(name="mm", bufs=3, space="PSUM") as psum,
            tc.tile_pool(name="sc", bufs=3, space="PSUM") as spsum,
            tc.tile_pool(name="tp", bufs=2, space="PSUM") as tpsum,
            tc.tile_pool(name="tb", bufs=6) as tp,
            tc.tile_pool(name="jk", bufs=1) as jp,
            tc.tile_pool(name="al", bufs=2) as alp,
            tc.tile_pool(name="ms", bufs=1) as ms,
        ):
            # ---------------- prep: constants & weights ----------------
            eye = const.tile([P, P], f32, tag="eye")
            nc.sync.dma_start(eye[:], eye_dram[:, :])
            eye_bf = const.tile([P, P], bf16, tag="eye_bf")
            nc.vector.tensor_copy(eye_bf[:], eye[:])

            w1bf = []
            for e in range(NE):
                t = const.tile([P, ATT], bf16, tag=f"w1_{e}")
                nc.gpsimd.dma_start(t[:], w1[P * e : P * (e + 1), :])
                w1bf.append(t)

            w2t = []
            for e in range(ND):
                t = const.tile([P, ATT], bf16, tag=f"w2_{e}")
                nc.gpsimd.dma_start(t[:], w2[P * e : P * (e + 1), :])
                w2t.append(t)

            # hT_all[p, c, x] = hid[x, 128c + p] via natural load + PE transpose
            h_nat = ms.tile([X, DEC], f32, tag="h_nat")
            nc.sync.dma_start(h_nat[:], hid[:, :])
            hn_bf = ms.tile([X, DEC], bf16, tag="hn_bf")
            nc.vector.tensor_copy(hn_bf[:], h_nat[:])
            hTb = ms.tile([P, ND, X], bf16, tag="hTb")
            for c in range(ND):
                tps_h = tpsum.tile([P, X], bf16, tag="tp", name=f"tpsh{c}")
                nc.tensor.transpose(tps_h[:], hn_bf[:, P * c : P * (c + 1)], eye_bf[0:X, 0:X])
                nc.vector.tensor_copy(hTb[:, c, :], tps_h[:])

            # bias vectors: natural load, PE-transpose each into [128, NA],
            # then add the two bias transposes (both at partition 0).
            def load_transposed_vec(src_dram, name, dt):
                nat = ms.tile([1, ATT], f32, tag="bvec", name=f"nat_{name}", bufs=2)
                nc.sync.dma_start(nat[:], src_dram[None, :])
                tps_v = tpsum.tile([P, NA], f32, tag="tp", name=f"tps_{name}")
                for c in range(NA):
                    nc.tensor.transpose(
                        tps_v[:, c : c + 1], nat[:, P * c : P * (c + 1)], eye[0:1, 0:1]
                    )
                dst = ms.tile([P, NA], dt, tag=name, name=name)
                nc.vector.tensor_copy(dst[:], tps_v[:])
                return dst

            b1T = load_transposed_vec(w1b, "b1T", f32)
            b2T = load_transposed_vec(w2b, "b2T", f32)
            vwbf = load_transposed_vec(vw, "vwbf", bf16)
            bT = ms.tile([P, NA], f32, tag="bT")
            nc.vector.tensor_add(bT[:], b1T[:], b2T[:])

            # proj_h transposed, plus bias: phb[p, a, x]
            phb = ms.tile([P, NA, X], f32, tag="phb")
            for a in range(NA):
                ph_ps = psum.tile([P, X], f32, tag="mm")
                for e in range(ND):
                    nc.tensor.matmul(
                        ph_ps[:],
                        w2t[e][:, P * a : P * (a + 1)],
                        hTb[:, e, :],
                        start=(e == 0),
                        stop=(e == ND - 1),
                    )
                nc.vector.tensor_scalar_add(phb[:, a, :], ph_ps[:], bT[:, a : a + 1])


            # ---------------- per-example staging ----------------
            def emit_load_convert(x):
                # f32 loads split 4-ways per l-chunk to spread queues,
                # then DVE bf16 convert.
                fbs = []
                for c in range(8):
                    fnat = fnp.tile([P, ENC], f32, tag="fn")
                    for q in range(4):
                        nc.sync.dma_start(
                            fnat[:, 256 * q : 256 * (q + 1)],
                            feats[x, P * c : P * (c + 1), 256 * q : 256 * (q + 1)],
                        )
                    fb = fbp.tile([P, ENC], bf16, tag="fb")
                    nc.vector.tensor_copy(fb[:], fnat[:])
                    fbs.append(fb)
                return fbs

            def alloc_ft():
                return ms_ft()

            def ms_ft():
                t = ftp.tile([P, NE * L], bf16, tag="ft")
                return t

            def ft_view(ft):
                return ft.rearrange("p (e lc c) -> p e lc c", e=NE, lc=NE)

            def emit_transpose_group(fbs, ft, lc, h):
                # transpose blocks (e in [4h, 4h+4), l-chunk lc) of the
                # natural bf16 tiles into ft columns e*L + 128*lc.
                tps = tpsum.tile([P, 4 * P], bf16, tag="tp")
                for j in range(4):
                    e = 4 * h + j
                    nc.tensor.transpose(
                        tps[:, P * j : P * (j + 1)],
                        fbs[lc][:, P * e : P * (e + 1)],
                        eye_bf[:],
                    )
                dst = ft_view(ft)[:, 4 * h : 4 * h + 4, lc, :]
                src = tps.rearrange("p (e c) -> p e c", e=4)
                nc.vector.tensor_copy(dst, src)

            # ---------------- main per-example pipeline ----------------
            # V-dot matmuls trail the tanh by one block so the PE never
            # waits on ACT.
            pending = []

            def flush_pending():
                for sc_ap, vw_ap, tb_ap, st, sp in pending:
                    nc.tensor.matmul(sc_ap, vw_ap, tb_ap, start=st, stop=sp)
                pending.clear()

            # prologue: stage examples 0 and 1
            fb_map = {0: emit_load_convert(0)}
            ft_map = {0: alloc_ft()}
            for lc in range(NE):
                for h in range(2):
                    emit_transpose_group(fb_map[0], ft_map[0], lc, h)
            fb_map[1] = emit_load_convert(1)

            for x in range(X):
                if x + 2 < X:
                    fb_map[x + 2] = emit_load_convert(x + 2)
                if x + 1 < X:
                    ft_map[x + 1] = alloc_ft()
                    tgroups = [(lc, h) for lc in range(NE) for h in range(2)]
                else:
                    tgroups = []
                ft = ft_map[x]
                ftv = ft_view(ft)

                sc_h = {}
                mb = 0  # micro-block index
                for a in range(NA):
                    for lh in range(NL):
                        pp = psum.tile([P, LH], f32, tag="mm")
                        for e in range(NE):
                            nc.tensor.matmul(
                                pp[:],
                                w1bf[e][:, P * a : P * (a + 1)],
                                ft[:, e * L + LH * lh : e * L + LH * (lh + 1)],
                                start=(e == 0),
                                stop=(e == NE - 1),
                            )
                        flush_pending()
                        # stage next example's transposes between MM blocks
                        if tgroups and mb >= 2:
                            lc, h = tgroups.pop(0)
                            emit_transpose_group(fb_map[x + 1], ft_map[x + 1], lc, h)
                        mb += 1
                        if lh not in sc_h:
                            sc_h[lh] = spsum.tile([1, LH], f32, tag="sc", name=f"sch{lh}")
                        tb = tp.tile([P, LH], bf16, tag="tb")
                        nc.scalar.activation(tb[:], pp[:], Tanh, bias=phb[:, a, x : x + 1])
                        pending.append(
                            (sc_h[lh][:], vwbf[:, a : a + 1], tb[:], a == 0, a == NA - 1)
                        )

                flush_pending()
                for lc, h in tgroups:
                    emit_transpose_group(fb_map[x + 1], ft_map[x + 1], lc, h)
                fb_map.pop(x, None)

                # softmax over L on partition 0
                scores = alp.tile([1, L], f32, tag="scores")
                nc.vector.tensor_copy(scores[:, 0:LH], sc_h[0][:])
                nc.vector.tensor_copy(scores[:, LH:L], sc_h[1][:])
                negm = alp.tile([1, 1], f32, tag="negm")
                nc.vector.tensor_reduce(
                    negm[:], scores[:], axis=AX, op=mybir.AluOpType.max, negate=True
                )
                esb = alp.tile([1, L], f32, tag="esb")
                ssum = alp.tile([1, 1], f32, tag="ssum")
                nc.scalar.activation(esb[:], scores[:], Exp, bias=negm[:], accum_out=ssum[:])
                rinv = alp.tile([1, 1], f32, tag="rinv")
                nc.vector.reciprocal(rinv[:], ssum[:])
                a32 = alp.tile([1, L], f32, tag="a32")
                nc.vector.tensor_scalar_mul(a32[:], esb[:], rinv[:])
                nc.sync.dma_start(alpha_o[x, :], a32[:])
                abf = alp.tile([1, L], bf16, tag="abf")
                nc.vector.tensor_scalar_mul(abf[:], esb[:], rinv[:])
                arep = alp.tile([P, L], bf16, tag="arep")
                nc.gpsimd.partition_broadcast(arep[:], abf[:])

                # context: ctx[e-chunk] = sum_l fT[e][:, l] * alpha[l]
                ctx_x = alp.tile([P, NE], f32, tag="ctx_x")
                for e in range(NE):
                    jk = jp.tile([P, L], f32, tag="jk")
                    nc.vector.scalar_tensor_tensor(
                        out=jk[:],
                        in0=ft[:, e * L : (e + 1) * L],
                        scalar=1.0,
                        in1=arep[:],
                        op0=mult,
                        op1=mult,
                        accum_out=ctx_x[:, e : e + 1],
                    )
                # transpose [128, 8] -> [8, 128] and ship context[x] out
                ct_ps = tpsum.tile([X, P], f32, tag="tp", name=f"ctps{x}")
                nc.tensor.transpose(ct_ps[:], ctx_x[:], eye[:])
                ctr = alp.tile([X, P], f32, tag="ctr")
                nc.vector.tensor_copy(ctr[:], ct_ps[:])
                nc.sync.dma_start(ctx_o.rearrange("x (e c) -> x e c", e=NE)[x], ctr[:])


    nc.compile()
    return nc


def kernel(features, hidden_state, W1_w, W1_b, W2_w, W2_b, V_w, V_b):
    from concourse.bass_utils import run_bass_kernel_spmd

    if "nc" not in _CACHE:
        _CACHE["nc"] = _build()
    nc = _CACHE["nc"]

    features = np.ascontiguousarray(np.asarray(features, dtype=np.float32))
    hidden_state = np.ascontiguousarray(np.asarray(hidden_state, dtype=np.float32))
    W1_w = np.ascontiguousarray(np.asarray(W1_w, dtype=np.float32))
    W1_b = np.ascontiguousarray(np.asarray(W1_b, dtype=np.float32))
    W2_w = np.ascontiguousarray(np.asarray(W2_w, dtype=np.float32))
    W2_b = np.ascontiguousarray(np.asarray(W2_b, dtype=np.float32))
    V_w = np.ascontiguousarray(np.asarray(V_w, dtype=np.float32))

    in_maps = []
    for c in range(N_CORES):
        in_maps.append(
            {
                "features": np.ascontiguousarray(features[c * X : (c + 1) * X]),
                "hidden_state": np.ascontiguousarray(hidden_state[c * X : (c + 1) * X]),
                "W1_w": W1_w,
                "W1_b": W1_b,
                "W2_w": W2_w,
                "W2_b": W2_b,
                "V_w": V_w,
            }
        )

    res = run_bass_kernel_spmd(nc, in_maps, list(range(N_CORES)), **_CACHE.get("run_kwargs", {}))
    _CACHE["last_result"] = res
    alpha = np.concatenate([res.results[c]["alpha"] for c in range(N_CORES)], axis=0)
    context = np.concatenate([res.results[c]["context"] for c in range(N_CORES)], axis=0)
    return alpha, context
